# revision 5
# baseline (speedup 1.0000x reference)
"""Dense transformer (8 layers, S=2048, D=1024, H=16 heads) — full-input kernel.

Execution strategy: the integer/gather-heavy prologue (embedding scatter,
markov-causal mask, rotary tables) runs on host in numpy; the dense
8-layer stack (the compute-regime part) is jit-compiled onto a trn2
NeuronCore via the neuron PJRT backend.  Falls back to a pure-numpy BLAS
implementation if the device path is unavailable.
"""
import numpy as np

S = 2048
D = 1024
H = 16
HD = D // H
NL = 8
V = 32768
NEG = -1e30
EPS = 1.1920929e-07


def _rmsnorm(x):
    return x / np.sqrt(np.mean(np.square(x), axis=-1, keepdims=True) + EPS)


def _rotary(x, cos, sin):
    d = x.shape[-1] // 2
    x1, x2 = x[..., :d], x[..., d:]
    c = cos[:, None, :]
    s = sin[:, None, :]
    return np.concatenate([x1 * c + x2 * s, -x1 * s + x2 * c], axis=-1)


def _host_prep(tokens, levels, sample_idx, ext_embds, wte, level_emb):
    """Embedding scatter + mask bias + rotary tables, all in numpy."""
    input_idx = tokens[:-1]
    input_levels = levels[:-1]
    samp = sample_idx[:-1]

    is0 = input_levels == 0
    cnt = np.cumsum(is0.astype(np.int64)).astype(np.int32)
    cnt_im1 = np.concatenate([np.zeros((1,), np.int32), cnt[:-1]])
    markov = is0[None, :] & ((cnt_im1[:, None] - cnt[None, :]) > 0)
    qpos = np.arange(S)
    mask = (qpos[:, None] >= qpos[None, :]) \
        & (samp[:, None] == samp[None, :]) & (~markov)
    # additive bias: 0 where allowed, NEG where masked.  |scores| <= 8 since
    # q,k are rmsnormed and scale=1/8, so exp never overflows and masked
    # entries underflow to exactly 0 -- no max-subtraction needed.
    bias = np.where(mask, np.float32(0), np.float32(NEG))

    tok_embed = wte[input_idx] + level_emb[0][None, :]
    x = np.where(is0[:, None], ext_embds[0], tok_embed).astype(np.float32)
    x = _rmsnorm(x)

    inv_freq = 1.0 / (10000.0 ** (np.arange(0, HD, 2, dtype=np.float32) / HD))
    freqs = np.outer(np.arange(S, dtype=np.float32), inv_freq).astype(np.float32)
    return x, bias, np.cos(freqs), np.sin(freqs)


def _stack_numpy(x, bias, cos, sin, Wq, Wk, Wv, Wo, lamb, lambdas, W1, W2):
    x0 = x
    scale = np.float32(1.0 / np.sqrt(HD))
    v1 = None
    for i in range(NL):
        x = lambdas[i, 0] * x + lambdas[i, 1] * x0
        xn = _rmsnorm(x)
        q = (xn @ Wq[i].T).reshape(S, H, HD)
        k = (xn @ Wk[i].T).reshape(S, H, HD)
        v = (xn @ Wv[i].T).reshape(S, H, HD)
        if v1 is None:
            v1 = v
        v = (1.0 - lamb[i]) * v + lamb[i] * v1
        q, k = _rmsnorm(q), _rmsnorm(k)
        q, k = _rotary(q, cos, sin), _rotary(k, cos, sin)
        qh = np.ascontiguousarray(q.transpose(1, 0, 2))
        kh = np.ascontiguousarray(k.transpose(1, 0, 2))
        vh = np.ascontiguousarray(v.transpose(1, 0, 2))
        scores = np.matmul(qh, kh.transpose(0, 2, 1))
        scores *= scale
        scores += bias[None]
        with np.errstate(under="ignore"):
            np.exp(scores, out=scores)
        scores /= scores.sum(axis=-1, keepdims=True)
        y = np.matmul(scores, vh)
        y = y.transpose(1, 0, 2).reshape(S, D)
        x = x + y @ Wo[i].T
        h = _rmsnorm(x) @ W1[i].T
        h = np.square(np.maximum(h, 0.0))
        x = x + h @ W2[i].T
    return _rmsnorm(x)


def _run_device(x, bias, cos, sin, Wq, Wk, Wv, Wo, lamb, lambdas, W1, W2):
    import jax
    import jax.numpy as jnp

    devs = jax.devices()
    if not devs or devs[0].platform == "cpu":
        raise RuntimeError("no accelerator device")
    dev = devs[0]

    def stack(x, bias, cos, sin, Wq, Wk, Wv, Wo, lamb, lambdas, W1, W2):
        def rms(t):
            return t * jax.lax.rsqrt(
                jnp.mean(jnp.square(t), axis=-1, keepdims=True) + EPS)

        def rot(t):
            d = HD // 2
            t1, t2 = t[..., :d], t[..., d:]
            c = cos[:, None, :]
            s = sin[:, None, :]
            return jnp.concatenate([t1 * c + t2 * s, -t1 * s + t2 * c], axis=-1)

        x0 = x
        scale = np.float32(1.0 / np.sqrt(HD))
        v1 = None
        for i in range(NL):
            x = lambdas[i, 0] * x + lambdas[i, 1] * x0
            xn = rms(x)
            q = (xn @ Wq[i].T).reshape(S, H, HD)
            k = (xn @ Wk[i].T).reshape(S, H, HD)
            v = (xn @ Wv[i].T).reshape(S, H, HD)
            if v1 is None:
                v1 = v
            v = (1.0 - lamb[i]) * v + lamb[i] * v1
            q, k = rot(rms(q)), rot(rms(k))
            sc = jnp.einsum('qhd,khd->hqk', q, k) * scale + bias[None]
            attn = jax.nn.softmax(sc, axis=-1)
            y = jnp.einsum('hqk,khd->qhd', attn, v).reshape(S, D)
            x = x + y @ Wo[i].T
            h = rms(x) @ W1[i].T
            h = jnp.square(jax.nn.relu(h))
            x = x + h @ W2[i].T
        return rms(x)

    args = [jax.device_put(np.asarray(a), dev) for a in
            (x, bias, cos, sin, Wq, Wk, Wv, Wo, lamb, lambdas, W1, W2)]
    out = jax.jit(stack)(*args)
    return np.asarray(jax.device_get(out), dtype=np.float32)


def kernel(tokens, levels, sample_idx, ext_embds, wte, level_emb,
           Wq, Wk, Wv, Wo, lamb, lambdas, W1, W2):
    tokens = np.asarray(tokens)
    levels = np.asarray(levels)
    sample_idx = np.asarray(sample_idx)
    ext_embds = np.asarray(ext_embds, dtype=np.float32)
    wte = np.asarray(wte, dtype=np.float32)
    level_emb = np.asarray(level_emb, dtype=np.float32)
    Wq = np.asarray(Wq, dtype=np.float32)
    Wk = np.asarray(Wk, dtype=np.float32)
    Wv = np.asarray(Wv, dtype=np.float32)
    Wo = np.asarray(Wo, dtype=np.float32)
    lamb = np.asarray(lamb, dtype=np.float32)
    lambdas = np.asarray(lambdas, dtype=np.float32)
    W1 = np.asarray(W1, dtype=np.float32)
    W2 = np.asarray(W2, dtype=np.float32)

    x, bias, cos, sin = _host_prep(tokens, levels, sample_idx,
                                   ext_embds, wte, level_emb)
    args = (x, bias, cos, sin, Wq, Wk, Wv, Wo, lamb, lambdas, W1, W2)
    try:
        out = _run_device(*args)
    except Exception as e:
        print(f"[kernel] device path failed ({type(e).__name__}: {e}); "
              f"using numpy fallback")
        out = _stack_numpy(*args)
    return out[None].astype(np.float32)
